# revision 31
# baseline (speedup 1.0000x reference)
"""DAHead (dual attention head: PAM + CAM) Trainium2 Bass kernel.

Sharding: 8 cores = (batch b, query-half h); core = 2*b + h.
Each core receives ONLY its own 2048-column half of its sample in bf16
(the minimal host->device payload), projects q/k/v for that half, then the
core pair exchanges k/v on-device (AllReduce-add with shipped 0/1 placement
masks -- pure SPMD, no runtime offsets) so each core has the full key range
for its PAM half. Weights ship as 1/8 column shards and are reassembled the
same way with an 8-core AllReduce. CAM Gram partials are summed with the
pairwise AllReduce as before. Output returns in bf16.

Math restructuring (inherited from the validated baseline, rel-l2 ~2e-3):
  - energy computed transposed, [j, i] layout, so softmax needs no transposes
    anywhere in PAM: P^T comes straight out of exp.
  - no max-subtraction in the PAM softmax (energy range is ~±10; exp is safe
    in fp32); normalization folded into the pam psum eviction via a
    partition-broadcast row.
  - v is computed directly transposed (vT = xf^T @ wv^T), the only form the
    PAM AV matmul needs.
  - attn/N, /C, gamma scalings and the v bias fold into host-precomputed
    per-channel constants (cb, g1, gc).
  - CAM softmax(max-G) == exp(minG-G)/sum: one reduce_min + one fused
    exp+rowsum activation per row block.

Walrus on TRN2 allows only ONE sync wait on (self-loading fp32) matmuls, so
the build keeps every matmul's dependencies to a single semaphore: dummy PE
"absorber" ops whenever a chunk's matmuls would otherwise wait on two
engines, and ACT-only PSUM eviction in phase B so bank-reuse WARs merge with
the exp RAW on one sem.

Host runner: the jitted SPMD executable is built once and cached; donated
output buffers ping-pong (the previous call's device-resident outputs are
donated as the next call's output storage -- the kernel writes every output
element, so their contents are irrelevant), eliminating the 33 MB host->
device zero upload every call.
"""

import sys
import numpy as np
import ml_dtypes

sys.path.insert(0, "/opt/trn_rl_repo")

from contextlib import ExitStack

import concourse.bass as bass
import concourse.bacc as bacc
import concourse.bass_isa as bass_isa
import concourse.tile as tile
from concourse import mybir
from concourse.masks import make_identity

F32 = mybir.dt.float32
BF16 = mybir.dt.bfloat16
FP8 = mybir.dt.float8e4
FP8E5 = mybir.dt.float8e5
NBF = ml_dtypes.bfloat16
AF = mybir.ActivationFunctionType

B, C, H, W = 4, 512, 64, 64
CI = C // 2
N = H * W          # 4096
HN = N // 2        # 2048 queries per core
P = 128
CT = C // P        # 4 channel tiles
QT = CI // P       # 2 q/k channel tiles
JT = N // P        # 32 key tiles (full range)
JH = JT // 2       # 16 key tiles in own half
ICH = HN // 512    # 4 query chunks of 512
SC = float(1.0 / np.sqrt(np.float32(C)))
SN = float(1.0 / np.sqrt(np.float32(N)))

# packed weight columns: [wq 1024 | wk 1024 | wv 2048], 1/8 shard = 512 cols
WCOLS = CT * CI * 2 + CT * C   # 4096
WSH = WCOLS // 8               # 512
KVC = QT * HN + JH * C         # 4096 + 8192 = 12288 bf16 cols in kv exchange
# cst columns: bq(QT) bk(QT) cb(CT) g1 gc xsc(CT) km(2) wm(8)
CB_OFF = 2 * QT
G1_OFF = CB_OFF + CT
GC_OFF = G1_OFF + 1
XS_OFF = GC_OFF + 1
KM_OFF = XS_OFF + CT
WM_OFF = KM_OFF + 2
NCST = WM_OFF + 8              # 24
OUTC = CT * HN + 16            # int8 data + 4 bitcast f32 row scales
# single packed input buffer (int8 bytes per partition):
#   [ x int8 CT*HN | wsh bf16 WSH*2 | cst f32 NCST*4 ]
XB_OFF = CT * HN               # 8192
WB_OFF = XB_OFF                # weight shard bytes at 8192
CB_BYTE = WB_OFF + WSH * 2     # cst bytes at 9216 (4-byte aligned)
INC = CB_BYTE + NCST * 4       # 9312 bytes per partition

_CACHE: dict = {}


def _build_bass():
    nc = bacc.Bacc("TRN2", target_bir_lowering=False, debug=False,
                   num_devices=8)
    I8 = mybir.dt.int8
    inp = nc.declare_dram_parameter("inp", [P, INC], I8, isOutput=False)
    xh = inp[:, 0:XB_OFF].rearrange("p (a b) -> p a b", a=CT)  # [P, CT, HN] i8
    wsh = inp[:, WB_OFF:CB_BYTE].bitcast(BF16)                 # [P, WSH] bf16
    cst = inp[:, CB_BYTE:INC].bitcast(F32)                     # [P, NCST] f32
    # replicated output: all 8 cores' [P, OUTC] blocks (data + bitcast f32
    # scales in the 16-byte tail), assembled on-device by an AllGather so the
    # host fetches ONE shard instead of eight
    outp = nc.declare_dram_parameter("out", [8, P, OUTC], I8, isOutput=True)
    og_in = nc.dram_tensor("og_in", [P, OUTC], I8)
    og_out = nc.dram_tensor("og_out", [8, P, OUTC], I8, addr_space="Shared")
    wg_in = nc.dram_tensor("wg_in", [P, 8, WSH], BF16)
    wg_out = nc.dram_tensor("wg_out", [P, 8, WSH], BF16)
    kvg_in = nc.dram_tensor("kvg_in", [P, 2, KVC], BF16)
    kvg_out = nc.dram_tensor("kvg_out", [P, 2, KVC], BF16)
    g_in = nc.dram_tensor("g_in", [P, CT, C], BF16)
    g_out = nc.dram_tensor("g_out", [P, CT, C], BF16)

    PAIRS = [[0, 1], [2, 3], [4, 5], [6, 7]]
    ALL8 = [[0, 1, 2, 3, 4, 5, 6, 7]]

    with tile.TileContext(nc) as tc, ExitStack() as ctx:
        consts = ctx.enter_context(tc.tile_pool(name="consts", bufs=1))
        sap = ctx.enter_context(tc.tile_pool(name="sap", bufs=1))
        wpool = ctx.enter_context(tc.tile_pool(name="wpool", bufs=1))

        ident = consts.tile([P, P], F32)
        make_identity(nc, ident)
        identB = consts.tile([P, P], BF16)
        make_identity(nc, identB)

        cst_sb = consts.tile([P, NCST], F32)
        nc.sync.dma_start(out=cst_sb, in_=cst)
        bq_sb = cst_sb[:, 0:QT]
        bk_sb = cst_sb[:, QT:2 * QT]
        cb_sb = cst_sb[:, CB_OFF:CB_OFF + CT]
        g1_sb = cst_sb[:, G1_OFF:G1_OFF + 1]
        gc_sb = cst_sb[:, GC_OFF:GC_OFF + 1]
        xs_sb = cst_sb[:, XS_OFF:XS_OFF + CT]

        # ---- weight reassembly: masked place own shard, AllReduce over 8 ----
        wq_sb = wpool.tile([P, CT, CI], FP8)
        wk_sb = wpool.tile([P, CT, CI], FP8)
        wv_sb = wpool.tile([P, CT, C], FP8)
        with tc.tile_pool(name="wtmp", bufs=1) as wtmp:
            wsh_sb = wtmp.tile([P, WSH], BF16)
            nc.sync.dma_start(out=wsh_sb, in_=wsh)
            wstage = wtmp.tile([P, 8, WSH], BF16)
            for s in range(8):
                nc.vector.tensor_scalar_mul(
                    wstage[:, s, :], wsh_sb,
                    cst_sb[:, WM_OFF + s:WM_OFF + s + 1])
            nc.sync.dma_start(out=wg_in[:], in_=wstage)
            nc.gpsimd.collective_compute(
                "AllReduce", mybir.AluOpType.add, replica_groups=ALL8,
                ins=[wg_in[:].opt()], outs=[wg_out[:].opt()])
            wfull = wtmp.tile([P, 8, WSH], BF16)
            nc.sync.dma_start(out=wfull, in_=wg_out[:])
            nc.gpsimd.tensor_copy(
                wq_sb[:].rearrange("p a b -> p (a b)"),
                wfull[:, 0:2, :].rearrange("p a b -> p (a b)"))
            nc.gpsimd.tensor_copy(
                wk_sb[:].rearrange("p a b -> p (a b)"),
                wfull[:, 2:4, :].rearrange("p a b -> p (a b)"))
            nc.gpsimd.tensor_copy(
                wv_sb[:].rearrange("p a b -> p (a b)"),
                wfull[:, 4:8, :].rearrange("p a b -> p (a b)"))

        sa_sb = sap.tile([P, CT, HN], BF16)  # tanh(PAM) result, lives to the end

        with ExitStack() as ab:
            persist = ab.enter_context(tc.tile_pool(name="persist", bufs=1))
            q_sb = persist.tile([P, QT, HN], FP8)
            k_sb = persist.tile([P, QT, N], FP8)
            vT_sb = persist.tile([P, JT, C], FP8)

            # ---------------- phase A: own-half projections q, k, vT --------
            with tc.tile_pool(name="kvh", bufs=1) as kvhp, \
                 tc.tile_pool(name="stream", bufs=3) as stream, \
                 tc.tile_pool(name="psDummyA", bufs=1, space="PSUM") as psDA, \
                 tc.tile_pool(name="psA", bufs=2, space="PSUM") as psA:
                kvhalf = kvhp.tile([P, KVC], BF16)  # k then vT, own half, bf16
                dummy_mm = psDA.tile([1, P], F32)   # fp8 absorber target
                # absorb each weight tensor's Pool-cast wait with a 1-wait PE op
                for w in (wk_sb, wq_sb, wv_sb):
                    nc.tensor.matmul(dummy_mm, w[:, 0, 0:1], w[:, 0, 0:P],
                                     start=True, stop=True)
                for jch in range(ICH):  # 512-wide column chunks over own half
                    jsl = slice(jch * 512, (jch + 1) * 512)
                    sti = stream.tile([P, CT, 512], mybir.dt.int8, tag="xi8")
                    for kt in range(CT):
                        nc.sync.dma_start(out=sti[:, kt, :], in_=xh[:, kt, jsl])
                    # dequantize int8 -> fp8 in one ACT pass (per-channel scale);
                    # ACT also owns the k/q psum evictions, so the consuming
                    # matmuls keep a single (ACT) semaphore wait
                    st = stream.tile([P, CT, 512], FP8, tag="x8")
                    for kt in range(CT):
                        nc.scalar.activation(st[:, kt, :], sti[:, kt, :],
                                             AF.Identity,
                                             scale=xs_sb[:, kt:kt + 1])
                    for t in range(QT):
                        kp = psA.tile([P, 512], F32, tag="psKQ")
                        for m in range(2):
                            nc.tensor.matmul(
                                kp, wk_sb[:, 2 * m:2 * m + 2, t * P:(t + 1) * P],
                                st[:, 2 * m:2 * m + 2, :],
                                start=(m == 0), stop=(m == 1),
                                perf_mode=mybir.MatmulPerfMode.DoubleRow)
                        nc.scalar.activation(
                            kvhalf[:, t * HN + jch * 512:t * HN + (jch + 1) * 512],
                            kp, AF.Identity, bias=bk_sb[:, t:t + 1])
                        qp = psA.tile([P, 512], F32, tag="psKQ")
                        for m in range(2):
                            nc.tensor.matmul(
                                qp, wq_sb[:, 2 * m:2 * m + 2, t * P:(t + 1) * P],
                                st[:, 2 * m:2 * m + 2, :],
                                start=(m == 0), stop=(m == 1),
                                perf_mode=mybir.MatmulPerfMode.DoubleRow)
                        nc.scalar.activation(q_sb[:, t, jsl], qp, AF.Identity,
                                             bias=bq_sb[:, t:t + 1])
                    for nt in range(4):
                        vp = psA.tile([P, 512], F32, tag="psV")
                        for m in range(2):
                            nc.tensor.matmul(
                                vp, st[:, 2 * m:2 * m + 2, nt * P:(nt + 1) * P],
                                wv_sb[:, 2 * m:2 * m + 2, :],
                                start=(m == 0), stop=(m == 1),
                                perf_mode=mybir.MatmulPerfMode.DoubleRow)
                        nc.vector.tensor_copy(
                            kvhalf[:, QT * HN + (jch * 4 + nt) * C:
                                   QT * HN + (jch * 4 + nt + 1) * C], vp)

                # ---- pair exchange of k/vT (AllReduce-add + 0/1 masks) ----
                with tc.tile_pool(name="exch", bufs=1) as exch:
                    blk2 = exch.tile([P, 2, KVC], BF16)
                    for r in range(2):
                        nc.vector.tensor_scalar_mul(
                            blk2[:, r, :], kvhalf,
                            cst_sb[:, KM_OFF + r:KM_OFF + r + 1])
                    nc.sync.dma_start(out=kvg_in[:], in_=blk2)
                    nc.gpsimd.collective_compute(
                        "AllReduce", mybir.AluOpType.add, replica_groups=PAIRS,
                        ins=[kvg_in[:].opt()], outs=[kvg_out[:].opt()])
                    kvfull = exch.tile([P, 2, KVC], BF16)
                    nc.sync.dma_start(out=kvfull, in_=kvg_out[:])
                    for r in range(2):
                        for t in range(QT):
                            nc.gpsimd.tensor_copy(
                                k_sb[:, t, r * HN:(r + 1) * HN],
                                kvfull[:, r, t * HN:(t + 1) * HN])
                        nc.gpsimd.tensor_copy(
                            vT_sb[:, r * JH:(r + 1) * JH, :],
                            kvfull[:, r, QT * HN:KVC].rearrange(
                                "p (a b) -> p a b", a=JH))
                    # absorb the Pool casts of k_sb/vT_sb with a 1-wait PE op
                    # so phase B's first energy matmul keeps a single
                    # (ACT, q_sb) wait
                    nc.tensor.matmul(dummy_mm, vT_sb[:, 0, 0:1],
                                     vT_sb[:, 0, 0:P], start=True, stop=True)

            # ---------------- phase B: PAM attention ----------------
            NP2 = JT // 2  # 16 key-block pairs per query chunk
            with tc.tile_pool(name="ptpool", bufs=6) as ptp, \
                 tc.tile_pool(name="sst", bufs=2) as sst, \
                 tc.tile_pool(name="accB", bufs=2) as accB, \
                 tc.tile_pool(name="xres", bufs=1) as xres, \
                 tc.tile_pool(name="psE", bufs=2, space="PSUM") as psE, \
                 tc.tile_pool(name="psPam", bufs=1, space="PSUM") as psP:
                for ich in range(ICH):
                    isl = slice(ich * 512, (ich + 1) * 512)
                    xri = xres.tile([P, CT, 512], mybir.dt.int8, tag="xri")
                    nc.sync.dma_start(out=xri, in_=xh[:, :, isl])
                    # dequantize the residual on Pool (idle-ish here; keeps
                    # the ACT exp stream clean)
                    xr = xres.tile([P, CT, 512], BF16, tag="xr")
                    for ct in range(CT):
                        nc.gpsimd.tensor_scalar_mul(
                            xr[:, ct, :], xri[:, ct, :],
                            xs_sb[:, ct:ct + 1])
                    pam = [psP.tile([P, 512], F32, tag=f"pam{t}", name=f"pam{t}_{ich}")
                           for t in range(CT)]
                    acc_d = accB.tile([P, 2, 512], F32, tag="sacc_d")
                    acc_p = accB.tile([P, 2, 512], F32, tag="sacc_p")
                    for p2 in range(NP2):
                        ep2 = psE.tile([P, 2, 512], F32, tag="e")
                        for h in range(2):
                            jt = 2 * p2 + h
                            nc.tensor.matmul(
                                ep2[:, h, :], k_sb[:, :, jt * P:(jt + 1) * P],
                                q_sb[:, :, isl], start=True, stop=True,
                                perf_mode=mybir.MatmulPerfMode.DoubleRow)
                        pt8 = ptp.tile([P, 2, 512], FP8E5, tag="pt")
                        nc.scalar.activation(pt8, ep2, AF.Exp, scale=SC)
                        # exp-sum accumulation alternates DVE/Pool partial accs
                        eng, acc = ((nc.vector, acc_d) if p2 % 2 == 0
                                    else (nc.gpsimd, acc_p))
                        if p2 < 2:
                            eng.tensor_copy(acc, pt8)
                        else:
                            eng.tensor_add(acc, acc, pt8)
                        for ct in range(CT):
                            nc.tensor.matmul(
                                pam[ct], vT_sb[:, 2 * p2:2 * p2 + 2,
                                               ct * P:(ct + 1) * P],
                                pt8, start=(p2 == 0), stop=(p2 == NP2 - 1),
                                perf_mode=mybir.MatmulPerfMode.DoubleRow)
                    # free pam banks first (ACT: same sem as the pt8 RAW, so
                    # ich+1's group-start matmuls keep a single sync wait)
                    t0s = []
                    for ct in range(CT):
                        t0 = sst.tile([P, 512], BF16, tag=f"t0{ct}")
                        nc.scalar.activation(t0, pam[ct], AF.Copy)
                        t0s.append(t0)
                    s2 = sst.tile([P, 2, 512], F32, tag="s2")
                    nc.vector.tensor_add(s2, acc_d, acc_p)
                    sacc = sst.tile([P, 512], F32, tag="saccf")
                    nc.vector.tensor_add(sacc, s2[:, 0, :], s2[:, 1, :])
                    red = accB.tile([P, 512], F32, tag="red")
                    nc.gpsimd.partition_all_reduce(red, sacc, 128,
                                                   bass_isa.ReduceOp.add)
                    binv = sst.tile([P, 512], F32, tag="binv")
                    nc.vector.reciprocal(binv, red)
                    bcs = sst.tile([P, 512], F32, tag="bcs")
                    nc.vector.tensor_scalar_mul(bcs, binv, g1_sb[:, 0:1])
                    for ct in range(CT):
                        t1 = sst.tile([P, 512], BF16, tag="t1")
                        nc.vector.tensor_mul(t1, t0s[ct], bcs)
                        t2 = sst.tile([P, 512], BF16, tag="t2")
                        nc.vector.tensor_add(t2, t1, xr[:, ct, :])
                        nc.scalar.activation(sa_sb[:, ct, isl], t2, AF.Tanh,
                                             bias=cb_sb[:, ct:ct + 1])

        # ---------------- phase C: CAM ----------------
        with tc.tile_pool(name="phC", bufs=1) as phC, \
             tc.tile_pool(name="stg", bufs=3) as stg, \
             tc.tile_pool(name="psDummyC", bufs=1, space="PSUM") as psDC, \
             tc.tile_pool(name="psT", bufs=2, space="PSUM") as psT, \
             tc.tile_pool(name="psG", bufs=4, space="PSUM") as psG:
            dummy_psb = psDC.tile([P, P], BF16)  # absorber, never read
            # absorb phase-B's max ACT tick (last tanh slice) in one PE wait
            nc.tensor.transpose(dummy_psb, identB, identB)
            nc.tensor.transpose(dummy_psb, sa_sb[:, CT - 1, HN - P:HN], identB)
            saT_sb = phC.tile([P, HN // P, C], BF16)  # [128, 16, 512]
            for it in range(HN // P):
                for ct in range(CT):
                    tp = psT.tile([P, P], BF16, tag="tp")
                    nc.tensor.transpose(tp, sa_sb[:, ct, it * P:(it + 1) * P], identB)
                    dst = saT_sb[:, it, ct * P:(ct + 1) * P]
                    if (it * CT + ct) % 2 == 0:
                        nc.vector.tensor_copy(dst, tp)
                    else:
                        nc.scalar.activation(dst, tp, AF.Copy)
            nc.tensor.transpose(dummy_psb, saT_sb[:, 0, 0:P], identB)
            nc.tensor.transpose(dummy_psb, saT_sb[:, 0, P:2 * P], identB)
            gp_sb = phC.tile([P, CT, C], BF16)
            for ct in range(CT):
                gp = psG.tile([P, C], F32, tag="g")
                for it in range(HN // P):
                    nc.tensor.matmul(gp, saT_sb[:, it, ct * P:(ct + 1) * P],
                                     saT_sb[:, it, :],
                                     start=(it == 0), stop=(it == HN // P - 1))
                nc.vector.tensor_copy(gp_sb[:, ct, :], gp)
            nc.sync.dma_start(out=g_in[:], in_=gp_sb)
            nc.gpsimd.collective_compute(
                "AllReduce", mybir.AluOpType.add,
                replica_groups=[[0, 1], [2, 3], [4, 5], [6, 7]],
                ins=[g_in[:].opt()], outs=[g_out[:].opt()])
            g2_sb = phC.tile([P, CT, C], BF16)
            nc.sync.dma_start(out=g2_sb, in_=g_out[:])
            a_sb = phC.tile([P, CT, C], BF16)
            for ct in range(CT):
                m = stg.tile([P, 1], F32, tag="m")
                nc.vector.tensor_reduce(out=m, in_=g2_sb[:, ct, :],
                                        op=mybir.AluOpType.min,
                                        axis=mybir.AxisListType.X)
                msc = stg.tile([P, 1], F32, tag="msc")
                nc.vector.tensor_scalar_mul(msc, m, SN)
                s = stg.tile([P, 1], F32, tag="s")
                e = stg.tile([P, C], F32, tag="ec")
                nc.scalar.activation(e, g2_sb[:, ct, :], AF.Exp,
                                     bias=msc, scale=-SN, accum_out=s)
                invc = stg.tile([P, 1], F32, tag="invc")
                nc.vector.reciprocal(invc, s)
                nc.scalar.activation(a_sb[:, ct, :], e, AF.Identity, scale=invc)
            aT_sb = phC.tile([P, CT, C], BF16)
            for ct in range(CT):
                for dt in range(CT):
                    tp = psT.tile([P, P], BF16, tag="tp")
                    nc.tensor.transpose(tp, a_sb[:, ct, dt * P:(dt + 1) * P], identB)
                    nc.vector.tensor_copy(aT_sb[:, dt, ct * P:(ct + 1) * P], tp)
            osc_sb = phC.tile([P, CT], F32)
            o8full = phC.tile([P, OUTC], mybir.dt.int8)
            for ct in range(CT):
                oc = stg.tile([P, HN], BF16, tag="oc")
                for ich in range(ICH):
                    isl = slice(ich * 512, (ich + 1) * 512)
                    cp = psG.tile([P, 512], F32, tag="g")
                    for dt in range(CT):
                        nc.tensor.matmul(cp, aT_sb[:, dt, ct * P:(ct + 1) * P],
                                         sa_sb[:, dt, isl],
                                         start=(dt == 0), stop=(dt == CT - 1))
                    nc.vector.scalar_tensor_tensor(
                        oc[:, isl], cp, gc_sb[:, 0:1], sa_sb[:, ct, isl],
                        mybir.AluOpType.mult, mybir.AluOpType.add)
                # int8 row quantization: scale = 127/absmax, ship absmax/127
                am = stg.tile([P, 1], F32, tag="am")
                nc.vector.tensor_reduce(out=am, in_=oc,
                                        op=mybir.AluOpType.max,
                                        apply_absolute_value=True,
                                        axis=mybir.AxisListType.X)
                nc.vector.tensor_scalar_mul(osc_sb[:, ct:ct + 1], am,
                                            float(1.0 / 127.0))
                qs = stg.tile([P, 1], F32, tag="qs")
                nc.vector.reciprocal(qs, am)
                qs2 = stg.tile([P, 1], F32, tag="qs2")
                nc.vector.tensor_scalar_mul(qs2, qs, 127.0)
                nc.scalar.activation(o8full[:, ct * HN:(ct + 1) * HN], oc,
                                     AF.Identity, scale=qs2)
            # bitcast the four f32 row scales into the 16-byte int8 tail
            nc.vector.tensor_copy(o8full[:, CT * HN:OUTC].bitcast(F32), osc_sb)
            # gather every core's block on-device so the host reads 1 shard
            nc.sync.dma_start(out=og_in[:], in_=o8full)
            nc.gpsimd.collective_compute(
                "AllGather", mybir.AluOpType.bypass, replica_groups=ALL8,
                ins=[og_in[:].opt()], outs=[og_out[:].opt()])
            nc.sync.dma_start(out=outp[:], in_=og_out[:])
    nc.compile()
    return nc


def _get_nc():
    if "nc" not in _CACHE:
        _CACHE["nc"] = _build_bass()
    return _CACHE["nc"]


def _part(a2d, nt, dtype=np.float32):
    """[nt*128, F] -> [128, nt, F] contiguous (partition-major tiles)."""
    f = a2d.shape[1]
    return np.ascontiguousarray(
        a2d.reshape(nt, P, f).transpose(1, 0, 2).astype(dtype))


def _in_maps(x, wq, bq, wk, bk, wv, bv, gamma_pam, gamma_cam):
    gp = float(np.asarray(gamma_pam).reshape(-1)[0])
    gc = float(np.asarray(gamma_cam).reshape(-1)[0])
    wq_a = _part(np.asarray(wq, np.float32).T, CT, NBF).reshape(P, CT * CI)
    wk_a = _part(np.asarray(wk, np.float32).T, CT, NBF).reshape(P, CT * CI)
    wv_a = _part(np.asarray(wv, np.float32).T, CT, NBF).reshape(P, CT * C)
    wpacked = np.concatenate([wq_a, wk_a, wv_a], axis=1)  # [P, 4096] bf16
    bq_a = np.asarray(bq, np.float32).reshape(QT, P).T
    bk_a = np.asarray(bk, np.float32).reshape(QT, P).T
    cb_a = (gp * np.asarray(bv, np.float32) / N).reshape(CT, P).T
    xf = np.asarray(x, np.float32).reshape(B, C, N)
    # per-(sample, channel) int8 quantization of x (full-channel absmax so
    # both cores of a pair use the same scale)
    xamax = np.maximum(np.abs(xf).max(axis=2, keepdims=True), 1e-30)
    xq8 = np.rint(xf * (127.0 / xamax)).astype(np.int8)
    xsc = (xamax[:, :, 0] / 127.0).astype(np.float32)  # [B, C]
    maps = []
    for core in range(8):
        b, h = core // 2, core % 2
        cst = np.zeros((P, NCST), np.float32)
        cst[:, 0:QT] = bq_a
        cst[:, QT:2 * QT] = bk_a
        cst[:, CB_OFF:CB_OFF + CT] = cb_a
        cst[:, G1_OFF] = gp / N
        cst[:, GC_OFF] = gc / C
        cst[:, XS_OFF:XS_OFF + CT] = xsc[b].reshape(CT, P).T
        cst[:, KM_OFF + h] = 1.0
        cst[:, WM_OFF + core] = 1.0
        # pack [x int8 | wsh bf16 | cst f32] into one int8 row buffer
        buf = np.empty((P, INC), np.int8)
        buf[:, :XB_OFF] = _part(
            xq8[b][:, h * HN:(h + 1) * HN], CT, np.int8).reshape(P, XB_OFF)
        buf[:, WB_OFF:CB_BYTE] = np.ascontiguousarray(
            wpacked[:, core * WSH:(core + 1) * WSH]).view(np.int8)
        buf[:, CB_BYTE:INC] = cst.view(np.int8)
        maps.append({"inp": buf})
    return maps


def _get_rt():
    """Build (once) the cached SPMD runtime: jitted shard_map executable."""
    if "rt" in _CACHE:
        return _CACHE["rt"]
    import jax
    from jax.sharding import Mesh, PartitionSpec
    from jax.experimental.shard_map import shard_map
    from concourse.bass2jax import (_bass_exec_p, install_neuronx_cc_hook,
                                    partition_id_tensor)

    nc = _get_nc()
    install_neuronx_cc_hook()
    partition_name = (nc.partition_id_tensor.name
                      if nc.partition_id_tensor else None)
    in_names, out_names, out_avals = [], [], []
    for alloc in nc.m.functions[0].allocations:
        if not isinstance(alloc, mybir.MemoryLocationSet):
            continue
        name = alloc.memorylocations[0].name
        if alloc.kind == "ExternalInput":
            if name != partition_name:
                in_names.append(name)
        elif alloc.kind == "ExternalOutput":
            out_names.append(name)
            out_avals.append(jax.core.ShapedArray(
                tuple(alloc.tensor_shape), mybir.dt.np(alloc.dtype)))
    n_params = len(in_names)
    n_outs = len(out_names)
    in_names_all = (in_names + out_names
                    + ([partition_name] if partition_name else []))
    donate = tuple(range(n_params, n_params + n_outs))

    def _body(*args):
        operands = list(args)
        if partition_name is not None:
            operands.append(partition_id_tensor())
        outs = _bass_exec_p.bind(
            *operands, out_avals=tuple(out_avals),
            in_names=tuple(in_names_all), out_names=tuple(out_names),
            lowering_input_output_aliases=(),
            sim_require_finite=True, sim_require_nnan=True, nc=nc)
        return tuple(outs)

    devices = jax.devices()[:8]
    mesh = Mesh(np.asarray(devices), ("core",))
    # inputs are per-core sharded; the output is replicated (the kernel's
    # final AllGather makes every core hold the full result) so the host
    # fetches a single shard
    in_specs = ((PartitionSpec("core"),) * n_params
                + (PartitionSpec(),) * n_outs)
    out_specs = (PartitionSpec(),) * n_outs
    sharded = jax.jit(
        shard_map(_body, mesh=mesh, in_specs=in_specs,
                  out_specs=out_specs, check_rep=False),
        donate_argnums=donate, keep_unused=True)
    _CACHE["rt"] = {
        "jax": jax, "sharded": sharded, "in_names": in_names,
        "out_names": out_names, "out_avals": out_avals, "prev_outs": None,
    }
    return _CACHE["rt"]


def _run(in_maps, **kw):
    """One full SPMD dispatch: host inputs -> 8 cores -> host outputs.

    The jitted executable is cached across calls; the previous call's
    device-resident output buffers are donated as this call's output
    storage (the kernel writes every output element, so contents are
    irrelevant) -- the first call falls back to host zeros.
    """
    from types import SimpleNamespace
    rt = _get_rt()
    jax = rt["jax"]
    concat_in = [
        np.concatenate([np.asarray(m[name]) for m in in_maps], axis=0)
        for name in rt["in_names"]]
    prev = rt["prev_outs"]
    if prev is None:
        # replicated output buffers: global shape == per-core shape
        prev = [np.zeros(tuple(av.shape), av.dtype) for av in rt["out_avals"]]
    outs = rt["sharded"](*concat_in, *prev)
    np_outs = [np.asarray(o) for o in outs]
    rt["prev_outs"] = list(outs)
    results = []
    for core in range(8):
        d = {}
        for i, name in enumerate(rt["out_names"]):
            d[name] = np_outs[i][core]  # out[8, P, OUTC]: core's block
        results.append(d)
    return SimpleNamespace(results=results, exec_time_ns=None,
                           profile_json=None, instructions_and_trace=None)


def kernel(**inputs) -> np.ndarray:
    maps = _in_maps(**inputs)
    res = _run(maps).results
    out = np.zeros((B, C, N), np.float32)
    for core in range(8):
        b, h = core // 2, core % 2
        blk = np.asarray(res[core]["out"])                   # [128, OUTC] int8
        o8 = blk[:, :CT * HN].reshape(P, CT, HN)
        osc = blk[:, CT * HN:].copy().view(np.float32)       # [128, CT]
        o = o8.astype(np.float32) * osc[:, :, None]
        out[b][:, h * HN:(h + 1) * HN] = o.transpose(1, 0, 2).reshape(C, HN)
    return out.reshape(B, C, H, W)


# revision 33
# speedup vs baseline: 1.0161x; 1.0161x over previous
"""DAHead (dual attention head: PAM + CAM) Trainium2 Bass kernel.

Sharding: 8 cores = (batch b, query-half h); core = 2*b + h.
The wall clock here is dominated by the axon host<->device tunnel, so the
host I/O is minimized: each core receives ONLY its own 2048-column half of
its sample, int8-quantized per channel (dequantized on-chip), plus a 1/8
column shard of the packed weights -- one packed int8 buffer per core
(~1.2 MB). Cores project q/k/v from their own half, then the core pair
exchanges k/v on-device (AllReduce-add with shipped 0/1 placement masks --
pure SPMD, no runtime offsets) so each core has the full key range for its
PAM half; weights are reassembled the same way with an 8-core AllReduce.
CAM Gram partials are summed with the pairwise AllReduce as before. The
output is int8 row-quantized (f32 scales bitcast into a 16-byte tail) and
AllGathered on-device so the host fetches a single replicated ~8.4 MB
shard instead of eight.

Math restructuring (inherited from the validated baseline, rel-l2 ~2e-3):
  - energy computed transposed, [j, i] layout, so softmax needs no transposes
    anywhere in PAM: P^T comes straight out of exp.
  - no max-subtraction in the PAM softmax (energy range is ~±10; exp is safe
    in fp32); normalization folded into the pam psum eviction via a
    partition-broadcast row.
  - v is computed directly transposed (vT = xf^T @ wv^T), the only form the
    PAM AV matmul needs.
  - attn/N, /C, gamma scalings and the v bias fold into host-precomputed
    per-channel constants (cb, g1, gc).
  - CAM softmax(max-G) == exp(minG-G)/sum: one reduce_min + one fused
    exp+rowsum activation per row block.

Walrus on TRN2 allows only ONE sync wait on (self-loading fp32) matmuls, so
the build keeps every matmul's dependencies to a single semaphore: dummy PE
"absorber" ops whenever a chunk's matmuls would otherwise wait on two
engines, and ACT-only PSUM eviction in phase B so bank-reuse WARs merge with
the exp RAW on one sem.

Host runner: the jitted SPMD executable is built once and cached; donated
output buffers ping-pong (the previous call's device-resident outputs are
donated as the next call's output storage -- the kernel writes every output
element, so their contents are irrelevant), eliminating the 33 MB host->
device zero upload every call.
"""

import sys
import numpy as np
import ml_dtypes

sys.path.insert(0, "/opt/trn_rl_repo")

from contextlib import ExitStack

import concourse.bass as bass
import concourse.bacc as bacc
import concourse.bass_isa as bass_isa
import concourse.tile as tile
from concourse import mybir
from concourse.masks import make_identity

F32 = mybir.dt.float32
BF16 = mybir.dt.bfloat16
FP8 = mybir.dt.float8e4
FP8E5 = mybir.dt.float8e5
NBF = ml_dtypes.bfloat16
AF = mybir.ActivationFunctionType

B, C, H, W = 4, 512, 64, 64
CI = C // 2
N = H * W          # 4096
HN = N // 2        # 2048 queries per core
P = 128
CT = C // P        # 4 channel tiles
QT = CI // P       # 2 q/k channel tiles
JT = N // P        # 32 key tiles (full range)
JH = JT // 2       # 16 key tiles in own half
ICH = HN // 512    # 4 query chunks of 512
SC = float(1.0 / np.sqrt(np.float32(C)))
SN = float(1.0 / np.sqrt(np.float32(N)))

# packed weight columns: [wq 1024 | wk 1024 | wv 2048], 1/8 shard = 512 cols
WCOLS = CT * CI * 2 + CT * C   # 4096
WSH = WCOLS // 8               # 512
KVC = QT * HN + JH * C         # 4096 + 8192 = 12288 bf16 cols in kv exchange
# cst columns: bq(QT) bk(QT) cb(CT) g1 gc xsc(CT) km(2) wm(8)
CB_OFF = 2 * QT
G1_OFF = CB_OFF + CT
GC_OFF = G1_OFF + 1
XS_OFF = GC_OFF + 1
KM_OFF = XS_OFF + CT
WM_OFF = KM_OFF + 2
NCST = WM_OFF + 8              # 24
OUTC = CT * HN + 16            # int8 data + 4 bitcast f32 row scales
# single packed input buffer (int8 bytes per partition):
#   [ x int8 CT*HN | wsh bf16 WSH*2 | cst f32 NCST*4 ]
XB_OFF = CT * HN               # 8192
WB_OFF = XB_OFF                # weight shard bytes at 8192
CB_BYTE = WB_OFF + WSH * 2     # cst bytes at 9216 (4-byte aligned)
INC = CB_BYTE + NCST * 4       # 9312 bytes per partition

_CACHE: dict = {}


def _build_bass():
    nc = bacc.Bacc("TRN2", target_bir_lowering=False, debug=False,
                   num_devices=8)
    I8 = mybir.dt.int8
    inp = nc.declare_dram_parameter("inp", [P, INC], I8, isOutput=False)
    xh = inp[:, 0:XB_OFF].rearrange("p (a b) -> p a b", a=CT)  # [P, CT, HN] i8
    wsh = inp[:, WB_OFF:CB_BYTE].bitcast(BF16)                 # [P, WSH] bf16
    cst = inp[:, CB_BYTE:INC].bitcast(F32)                     # [P, NCST] f32
    # replicated output: all 8 cores' [P, OUTC] blocks (data + bitcast f32
    # scales in the 16-byte tail), assembled on-device by an AllGather so the
    # host fetches ONE shard instead of eight
    outp = nc.declare_dram_parameter("out", [8, P, OUTC], I8, isOutput=True)
    og_in = nc.dram_tensor("og_in", [P, OUTC], I8)
    og_out = nc.dram_tensor("og_out", [8, P, OUTC], I8, addr_space="Shared")
    wg_in = nc.dram_tensor("wg_in", [P, 8, WSH], BF16)
    wg_out = nc.dram_tensor("wg_out", [P, 8, WSH], BF16)
    kvg_in = nc.dram_tensor("kvg_in", [P, 2, KVC], BF16)
    kvg_out = nc.dram_tensor("kvg_out", [P, 2, KVC], BF16)
    g_in = nc.dram_tensor("g_in", [P, CT, C], BF16)
    g_out = nc.dram_tensor("g_out", [P, CT, C], BF16)

    PAIRS = [[0, 1], [2, 3], [4, 5], [6, 7]]
    ALL8 = [[0, 1, 2, 3, 4, 5, 6, 7]]

    with tile.TileContext(nc) as tc, ExitStack() as ctx:
        consts = ctx.enter_context(tc.tile_pool(name="consts", bufs=1))
        sap = ctx.enter_context(tc.tile_pool(name="sap", bufs=1))
        wpool = ctx.enter_context(tc.tile_pool(name="wpool", bufs=1))

        ident = consts.tile([P, P], F32)
        make_identity(nc, ident)
        identB = consts.tile([P, P], BF16)
        make_identity(nc, identB)

        cst_sb = consts.tile([P, NCST], F32)
        nc.sync.dma_start(out=cst_sb, in_=cst)
        bq_sb = cst_sb[:, 0:QT]
        bk_sb = cst_sb[:, QT:2 * QT]
        cb_sb = cst_sb[:, CB_OFF:CB_OFF + CT]
        g1_sb = cst_sb[:, G1_OFF:G1_OFF + 1]
        gc_sb = cst_sb[:, GC_OFF:GC_OFF + 1]
        xs_sb = cst_sb[:, XS_OFF:XS_OFF + CT]

        # ---- weight reassembly: masked place own shard, AllReduce over 8 ----
        wq_sb = wpool.tile([P, CT, CI], FP8)
        wk_sb = wpool.tile([P, CT, CI], FP8)
        wv_sb = wpool.tile([P, CT, C], FP8)
        with tc.tile_pool(name="wtmp", bufs=1) as wtmp:
            wsh_sb = wtmp.tile([P, WSH], BF16)
            nc.sync.dma_start(out=wsh_sb, in_=wsh)
            wstage = wtmp.tile([P, 8, WSH], BF16)
            for s in range(8):
                nc.vector.tensor_scalar_mul(
                    wstage[:, s, :], wsh_sb,
                    cst_sb[:, WM_OFF + s:WM_OFF + s + 1])
            nc.sync.dma_start(out=wg_in[:], in_=wstage)
            nc.gpsimd.collective_compute(
                "AllReduce", mybir.AluOpType.add, replica_groups=ALL8,
                ins=[wg_in[:].opt()], outs=[wg_out[:].opt()])
            wfull = wtmp.tile([P, 8, WSH], BF16)
            nc.sync.dma_start(out=wfull, in_=wg_out[:])
            nc.gpsimd.tensor_copy(
                wq_sb[:].rearrange("p a b -> p (a b)"),
                wfull[:, 0:2, :].rearrange("p a b -> p (a b)"))
            nc.gpsimd.tensor_copy(
                wk_sb[:].rearrange("p a b -> p (a b)"),
                wfull[:, 2:4, :].rearrange("p a b -> p (a b)"))
            nc.gpsimd.tensor_copy(
                wv_sb[:].rearrange("p a b -> p (a b)"),
                wfull[:, 4:8, :].rearrange("p a b -> p (a b)"))

        sa_sb = sap.tile([P, CT, HN], BF16)  # tanh(PAM) result, lives to the end

        with ExitStack() as ab:
            persist = ab.enter_context(tc.tile_pool(name="persist", bufs=1))
            q_sb = persist.tile([P, QT, HN], FP8)
            k_sb = persist.tile([P, QT, N], FP8)
            vT_sb = persist.tile([P, JT, C], FP8)

            # ---------------- phase A: own-half projections q, k, vT --------
            with tc.tile_pool(name="kvh", bufs=1) as kvhp, \
                 tc.tile_pool(name="stream", bufs=3) as stream, \
                 tc.tile_pool(name="psDummyA", bufs=1, space="PSUM") as psDA, \
                 tc.tile_pool(name="psA", bufs=2, space="PSUM") as psA:
                kvhalf = kvhp.tile([P, KVC], BF16)  # k then vT, own half, bf16
                dummy_mm = psDA.tile([1, P], F32)   # fp8 absorber target
                # absorb each weight tensor's Pool-cast wait with a 1-wait PE op
                for w in (wk_sb, wq_sb, wv_sb):
                    nc.tensor.matmul(dummy_mm, w[:, 0, 0:1], w[:, 0, 0:P],
                                     start=True, stop=True)
                for jch in range(ICH):  # 512-wide column chunks over own half
                    jsl = slice(jch * 512, (jch + 1) * 512)
                    sti = stream.tile([P, CT, 512], mybir.dt.int8, tag="xi8")
                    for kt in range(CT):
                        nc.sync.dma_start(out=sti[:, kt, :], in_=xh[:, kt, jsl])
                    # dequantize int8 -> fp8 in one ACT pass (per-channel scale);
                    # ACT also owns the k/q psum evictions, so the consuming
                    # matmuls keep a single (ACT) semaphore wait
                    st = stream.tile([P, CT, 512], FP8, tag="x8")
                    for kt in range(CT):
                        nc.scalar.activation(st[:, kt, :], sti[:, kt, :],
                                             AF.Identity,
                                             scale=xs_sb[:, kt:kt + 1])
                    for t in range(QT):
                        kp = psA.tile([P, 512], F32, tag="psKQ")
                        for m in range(2):
                            nc.tensor.matmul(
                                kp, wk_sb[:, 2 * m:2 * m + 2, t * P:(t + 1) * P],
                                st[:, 2 * m:2 * m + 2, :],
                                start=(m == 0), stop=(m == 1),
                                perf_mode=mybir.MatmulPerfMode.DoubleRow)
                        nc.scalar.activation(
                            kvhalf[:, t * HN + jch * 512:t * HN + (jch + 1) * 512],
                            kp, AF.Identity, bias=bk_sb[:, t:t + 1])
                        qp = psA.tile([P, 512], F32, tag="psKQ")
                        for m in range(2):
                            nc.tensor.matmul(
                                qp, wq_sb[:, 2 * m:2 * m + 2, t * P:(t + 1) * P],
                                st[:, 2 * m:2 * m + 2, :],
                                start=(m == 0), stop=(m == 1),
                                perf_mode=mybir.MatmulPerfMode.DoubleRow)
                        nc.scalar.activation(q_sb[:, t, jsl], qp, AF.Identity,
                                             bias=bq_sb[:, t:t + 1])
                    for nt in range(4):
                        vp = psA.tile([P, 512], F32, tag="psV")
                        for m in range(2):
                            nc.tensor.matmul(
                                vp, st[:, 2 * m:2 * m + 2, nt * P:(nt + 1) * P],
                                wv_sb[:, 2 * m:2 * m + 2, :],
                                start=(m == 0), stop=(m == 1),
                                perf_mode=mybir.MatmulPerfMode.DoubleRow)
                        nc.vector.tensor_copy(
                            kvhalf[:, QT * HN + (jch * 4 + nt) * C:
                                   QT * HN + (jch * 4 + nt + 1) * C], vp)

                # ---- pair exchange of k/vT (AllReduce-add + 0/1 masks) ----
                with tc.tile_pool(name="exch", bufs=1) as exch:
                    blk2 = exch.tile([P, 2, KVC], BF16)
                    for r in range(2):
                        nc.vector.tensor_scalar_mul(
                            blk2[:, r, :], kvhalf,
                            cst_sb[:, KM_OFF + r:KM_OFF + r + 1])
                    nc.sync.dma_start(out=kvg_in[:], in_=blk2)
                    nc.gpsimd.collective_compute(
                        "AllReduce", mybir.AluOpType.add, replica_groups=PAIRS,
                        ins=[kvg_in[:].opt()], outs=[kvg_out[:].opt()])
                    kvfull = exch.tile([P, 2, KVC], BF16)
                    nc.sync.dma_start(out=kvfull, in_=kvg_out[:])
                    for r in range(2):
                        for t in range(QT):
                            nc.gpsimd.tensor_copy(
                                k_sb[:, t, r * HN:(r + 1) * HN],
                                kvfull[:, r, t * HN:(t + 1) * HN])
                        nc.gpsimd.tensor_copy(
                            vT_sb[:, r * JH:(r + 1) * JH, :],
                            kvfull[:, r, QT * HN:KVC].rearrange(
                                "p (a b) -> p a b", a=JH))
                    # absorb the Pool casts of k_sb/vT_sb with a 1-wait PE op
                    # so phase B's first energy matmul keeps a single
                    # (ACT, q_sb) wait
                    nc.tensor.matmul(dummy_mm, vT_sb[:, 0, 0:1],
                                     vT_sb[:, 0, 0:P], start=True, stop=True)

            # ---------------- phase B: PAM attention ----------------
            NP2 = JT // 2  # 16 key-block pairs per query chunk
            with tc.tile_pool(name="ptpool", bufs=6) as ptp, \
                 tc.tile_pool(name="sst", bufs=2) as sst, \
                 tc.tile_pool(name="accB", bufs=2) as accB, \
                 tc.tile_pool(name="xres", bufs=1) as xres, \
                 tc.tile_pool(name="psE", bufs=2, space="PSUM") as psE, \
                 tc.tile_pool(name="psPam", bufs=1, space="PSUM") as psP:
                for ich in range(ICH):
                    isl = slice(ich * 512, (ich + 1) * 512)
                    xri = xres.tile([P, CT, 512], mybir.dt.int8, tag="xri")
                    nc.sync.dma_start(out=xri, in_=xh[:, :, isl])
                    # dequantize the residual on Pool (idle-ish here; keeps
                    # the ACT exp stream clean)
                    xr = xres.tile([P, CT, 512], BF16, tag="xr")
                    for ct in range(CT):
                        nc.gpsimd.tensor_scalar_mul(
                            xr[:, ct, :], xri[:, ct, :],
                            xs_sb[:, ct:ct + 1])
                    pam = [psP.tile([P, 512], F32, tag=f"pam{t}", name=f"pam{t}_{ich}")
                           for t in range(CT)]
                    acc_d = accB.tile([P, 2, 512], F32, tag="sacc_d")
                    acc_p = accB.tile([P, 2, 512], F32, tag="sacc_p")
                    for p2 in range(NP2):
                        ep2 = psE.tile([P, 2, 512], F32, tag="e")
                        for h in range(2):
                            jt = 2 * p2 + h
                            nc.tensor.matmul(
                                ep2[:, h, :], k_sb[:, :, jt * P:(jt + 1) * P],
                                q_sb[:, :, isl], start=True, stop=True,
                                perf_mode=mybir.MatmulPerfMode.DoubleRow)
                        pt8 = ptp.tile([P, 2, 512], FP8E5, tag="pt")
                        nc.scalar.activation(pt8, ep2, AF.Exp, scale=SC)
                        # exp-sum accumulation alternates DVE/Pool partial accs
                        eng, acc = ((nc.vector, acc_d) if p2 % 2 == 0
                                    else (nc.gpsimd, acc_p))
                        if p2 < 2:
                            eng.tensor_copy(acc, pt8)
                        else:
                            eng.tensor_add(acc, acc, pt8)
                        for ct in range(CT):
                            nc.tensor.matmul(
                                pam[ct], vT_sb[:, 2 * p2:2 * p2 + 2,
                                               ct * P:(ct + 1) * P],
                                pt8, start=(p2 == 0), stop=(p2 == NP2 - 1),
                                perf_mode=mybir.MatmulPerfMode.DoubleRow)
                    # free pam banks first (ACT: same sem as the pt8 RAW, so
                    # ich+1's group-start matmuls keep a single sync wait)
                    t0s = []
                    for ct in range(CT):
                        t0 = sst.tile([P, 512], BF16, tag=f"t0{ct}")
                        nc.scalar.activation(t0, pam[ct], AF.Copy)
                        t0s.append(t0)
                    s2 = sst.tile([P, 2, 512], F32, tag="s2")
                    nc.vector.tensor_add(s2, acc_d, acc_p)
                    sacc = sst.tile([P, 512], F32, tag="saccf")
                    nc.vector.tensor_add(sacc, s2[:, 0, :], s2[:, 1, :])
                    red = accB.tile([P, 512], F32, tag="red")
                    nc.gpsimd.partition_all_reduce(red, sacc, 128,
                                                   bass_isa.ReduceOp.add)
                    binv = sst.tile([P, 512], F32, tag="binv")
                    nc.vector.reciprocal(binv, red)
                    bcs = sst.tile([P, 512], F32, tag="bcs")
                    nc.vector.tensor_scalar_mul(bcs, binv, g1_sb[:, 0:1])
                    for ct in range(CT):
                        t1 = sst.tile([P, 512], BF16, tag="t1")
                        nc.vector.tensor_mul(t1, t0s[ct], bcs)
                        t2 = sst.tile([P, 512], BF16, tag="t2")
                        nc.vector.tensor_add(t2, t1, xr[:, ct, :])
                        nc.scalar.activation(sa_sb[:, ct, isl], t2, AF.Tanh,
                                             bias=cb_sb[:, ct:ct + 1])

        # ---------------- phase C: CAM ----------------
        with tc.tile_pool(name="phC", bufs=1) as phC, \
             tc.tile_pool(name="stg", bufs=3) as stg, \
             tc.tile_pool(name="psDummyC", bufs=1, space="PSUM") as psDC, \
             tc.tile_pool(name="psT", bufs=2, space="PSUM") as psT, \
             tc.tile_pool(name="psG", bufs=4, space="PSUM") as psG:
            dummy_psb = psDC.tile([P, P], BF16)  # absorber, never read
            # absorb phase-B's max ACT tick (last tanh slice) in one PE wait
            nc.tensor.transpose(dummy_psb, identB, identB)
            nc.tensor.transpose(dummy_psb, sa_sb[:, CT - 1, HN - P:HN], identB)
            saT_sb = phC.tile([P, HN // P, C], BF16)  # [128, 16, 512]
            for it in range(HN // P):
                for ct in range(CT):
                    tp = psT.tile([P, P], BF16, tag="tp")
                    nc.tensor.transpose(tp, sa_sb[:, ct, it * P:(it + 1) * P], identB)
                    dst = saT_sb[:, it, ct * P:(ct + 1) * P]
                    if (it * CT + ct) % 2 == 0:
                        nc.vector.tensor_copy(dst, tp)
                    else:
                        nc.scalar.activation(dst, tp, AF.Copy)
            nc.tensor.transpose(dummy_psb, saT_sb[:, 0, 0:P], identB)
            nc.tensor.transpose(dummy_psb, saT_sb[:, 0, P:2 * P], identB)
            gp_sb = phC.tile([P, CT, C], BF16)
            for ct in range(CT):
                gp = psG.tile([P, C], F32, tag="g")
                for it in range(HN // P):
                    nc.tensor.matmul(gp, saT_sb[:, it, ct * P:(ct + 1) * P],
                                     saT_sb[:, it, :],
                                     start=(it == 0), stop=(it == HN // P - 1))
                nc.vector.tensor_copy(gp_sb[:, ct, :], gp)
            nc.sync.dma_start(out=g_in[:], in_=gp_sb)
            nc.gpsimd.collective_compute(
                "AllReduce", mybir.AluOpType.add,
                replica_groups=[[0, 1], [2, 3], [4, 5], [6, 7]],
                ins=[g_in[:].opt()], outs=[g_out[:].opt()])
            g2_sb = phC.tile([P, CT, C], BF16)
            nc.sync.dma_start(out=g2_sb, in_=g_out[:])
            a_sb = phC.tile([P, CT, C], BF16)
            for ct in range(CT):
                m = stg.tile([P, 1], F32, tag="m")
                nc.vector.tensor_reduce(out=m, in_=g2_sb[:, ct, :],
                                        op=mybir.AluOpType.min,
                                        axis=mybir.AxisListType.X)
                msc = stg.tile([P, 1], F32, tag="msc")
                nc.vector.tensor_scalar_mul(msc, m, SN)
                s = stg.tile([P, 1], F32, tag="s")
                e = stg.tile([P, C], F32, tag="ec")
                nc.scalar.activation(e, g2_sb[:, ct, :], AF.Exp,
                                     bias=msc, scale=-SN, accum_out=s)
                invc = stg.tile([P, 1], F32, tag="invc")
                nc.vector.reciprocal(invc, s)
                nc.scalar.activation(a_sb[:, ct, :], e, AF.Identity, scale=invc)
            aT_sb = phC.tile([P, CT, C], BF16)
            for ct in range(CT):
                for dt in range(CT):
                    tp = psT.tile([P, P], BF16, tag="tp")
                    nc.tensor.transpose(tp, a_sb[:, ct, dt * P:(dt + 1) * P], identB)
                    nc.vector.tensor_copy(aT_sb[:, dt, ct * P:(ct + 1) * P], tp)
            osc_sb = phC.tile([P, CT], F32)
            o8full = phC.tile([P, OUTC], mybir.dt.int8)
            for ct in range(CT):
                oc = stg.tile([P, HN], BF16, tag="oc")
                for ich in range(ICH):
                    isl = slice(ich * 512, (ich + 1) * 512)
                    cp = psG.tile([P, 512], F32, tag="g")
                    for dt in range(CT):
                        nc.tensor.matmul(cp, aT_sb[:, dt, ct * P:(ct + 1) * P],
                                         sa_sb[:, dt, isl],
                                         start=(dt == 0), stop=(dt == CT - 1))
                    nc.vector.scalar_tensor_tensor(
                        oc[:, isl], cp, gc_sb[:, 0:1], sa_sb[:, ct, isl],
                        mybir.AluOpType.mult, mybir.AluOpType.add)
                # int8 row quantization: scale = 127/absmax, ship absmax/127
                am = stg.tile([P, 1], F32, tag="am")
                nc.vector.tensor_reduce(out=am, in_=oc,
                                        op=mybir.AluOpType.max,
                                        apply_absolute_value=True,
                                        axis=mybir.AxisListType.X)
                nc.vector.tensor_scalar_mul(osc_sb[:, ct:ct + 1], am,
                                            float(1.0 / 127.0))
                qs = stg.tile([P, 1], F32, tag="qs")
                nc.vector.reciprocal(qs, am)
                qs2 = stg.tile([P, 1], F32, tag="qs2")
                nc.vector.tensor_scalar_mul(qs2, qs, 127.0)
                nc.scalar.activation(o8full[:, ct * HN:(ct + 1) * HN], oc,
                                     AF.Identity, scale=qs2)
            # bitcast the four f32 row scales into the 16-byte int8 tail
            nc.vector.tensor_copy(o8full[:, CT * HN:OUTC].bitcast(F32), osc_sb)
            # gather every core's block on-device so the host reads 1 shard
            nc.sync.dma_start(out=og_in[:], in_=o8full)
            nc.gpsimd.collective_compute(
                "AllGather", mybir.AluOpType.bypass, replica_groups=ALL8,
                ins=[og_in[:].opt()], outs=[og_out[:].opt()])
            nc.sync.dma_start(out=outp[:], in_=og_out[:])
    nc.compile()
    return nc


def _get_nc():
    if "nc" not in _CACHE:
        _CACHE["nc"] = _build_bass()
    return _CACHE["nc"]


def _part(a2d, nt, dtype=np.float32):
    """[nt*128, F] -> [128, nt, F] contiguous (partition-major tiles)."""
    f = a2d.shape[1]
    return np.ascontiguousarray(
        a2d.reshape(nt, P, f).transpose(1, 0, 2).astype(dtype))


def _in_maps(x, wq, bq, wk, bk, wv, bv, gamma_pam, gamma_cam):
    gp = float(np.asarray(gamma_pam).reshape(-1)[0])
    gc = float(np.asarray(gamma_cam).reshape(-1)[0])
    wq_a = _part(np.asarray(wq, np.float32).T, CT, NBF).reshape(P, CT * CI)
    wk_a = _part(np.asarray(wk, np.float32).T, CT, NBF).reshape(P, CT * CI)
    wv_a = _part(np.asarray(wv, np.float32).T, CT, NBF).reshape(P, CT * C)
    wpacked = np.concatenate([wq_a, wk_a, wv_a], axis=1)  # [P, 4096] bf16
    bq_a = np.asarray(bq, np.float32).reshape(QT, P).T
    bk_a = np.asarray(bk, np.float32).reshape(QT, P).T
    cb_a = (gp * np.asarray(bv, np.float32) / N).reshape(CT, P).T
    xf = np.asarray(x, np.float32).reshape(B, C, N)
    # per-(sample, channel) int8 quantization of x (full-channel absmax so
    # both cores of a pair use the same scale)
    xamax = np.maximum(np.abs(xf).max(axis=2, keepdims=True), 1e-30)
    xq8 = np.rint(xf * (127.0 / xamax)).astype(np.int8)
    xsc = (xamax[:, :, 0] / 127.0).astype(np.float32)  # [B, C]
    maps = []
    for core in range(8):
        b, h = core // 2, core % 2
        cst = np.zeros((P, NCST), np.float32)
        cst[:, 0:QT] = bq_a
        cst[:, QT:2 * QT] = bk_a
        cst[:, CB_OFF:CB_OFF + CT] = cb_a
        cst[:, G1_OFF] = gp / N
        cst[:, GC_OFF] = gc / C
        cst[:, XS_OFF:XS_OFF + CT] = xsc[b].reshape(CT, P).T
        cst[:, KM_OFF + h] = 1.0
        cst[:, WM_OFF + core] = 1.0
        # pack [x int8 | wsh bf16 | cst f32] into one int8 row buffer
        buf = np.empty((P, INC), np.int8)
        buf[:, :XB_OFF] = _part(
            xq8[b][:, h * HN:(h + 1) * HN], CT, np.int8).reshape(P, XB_OFF)
        buf[:, WB_OFF:CB_BYTE] = np.ascontiguousarray(
            wpacked[:, core * WSH:(core + 1) * WSH]).view(np.int8)
        buf[:, CB_BYTE:INC] = cst.view(np.int8)
        maps.append({"inp": buf})
    return maps


def _get_rt():
    """Build (once) the cached SPMD runtime: jitted shard_map executable."""
    if "rt" in _CACHE:
        return _CACHE["rt"]
    import jax
    from jax.sharding import Mesh, PartitionSpec
    from jax.experimental.shard_map import shard_map
    from concourse.bass2jax import (_bass_exec_p, install_neuronx_cc_hook,
                                    partition_id_tensor)

    nc = _get_nc()
    install_neuronx_cc_hook()
    partition_name = (nc.partition_id_tensor.name
                      if nc.partition_id_tensor else None)
    in_names, out_names, out_avals = [], [], []
    for alloc in nc.m.functions[0].allocations:
        if not isinstance(alloc, mybir.MemoryLocationSet):
            continue
        name = alloc.memorylocations[0].name
        if alloc.kind == "ExternalInput":
            if name != partition_name:
                in_names.append(name)
        elif alloc.kind == "ExternalOutput":
            out_names.append(name)
            out_avals.append(jax.core.ShapedArray(
                tuple(alloc.tensor_shape), mybir.dt.np(alloc.dtype)))
    n_params = len(in_names)
    n_outs = len(out_names)
    in_names_all = (in_names + out_names
                    + ([partition_name] if partition_name else []))
    donate = tuple(range(n_params, n_params + n_outs))

    def _body(*args):
        operands = list(args)
        if partition_name is not None:
            operands.append(partition_id_tensor())
        outs = _bass_exec_p.bind(
            *operands, out_avals=tuple(out_avals),
            in_names=tuple(in_names_all), out_names=tuple(out_names),
            lowering_input_output_aliases=(),
            sim_require_finite=True, sim_require_nnan=True, nc=nc)
        return tuple(outs)

    devices = jax.devices()[:8]
    mesh = Mesh(np.asarray(devices), ("core",))
    # inputs are per-core sharded; the output is replicated (the kernel's
    # final AllGather makes every core hold the full result) so the host
    # fetches a single shard
    in_specs = ((PartitionSpec("core"),) * n_params
                + (PartitionSpec(),) * n_outs)
    out_specs = (PartitionSpec(),) * n_outs
    sharded = jax.jit(
        shard_map(_body, mesh=mesh, in_specs=in_specs,
                  out_specs=out_specs, check_rep=False),
        donate_argnums=donate, keep_unused=True)
    _CACHE["rt"] = {
        "jax": jax, "sharded": sharded, "in_names": in_names,
        "out_names": out_names, "out_avals": out_avals, "prev_outs": None,
    }
    return _CACHE["rt"]


def _run(in_maps, **kw):
    """One full SPMD dispatch: host inputs -> 8 cores -> host outputs.

    The jitted executable is cached across calls; the previous call's
    device-resident output buffers are donated as this call's output
    storage (the kernel writes every output element, so contents are
    irrelevant) -- the first call falls back to host zeros.
    """
    from types import SimpleNamespace
    rt = _get_rt()
    jax = rt["jax"]
    concat_in = [
        np.concatenate([np.asarray(m[name]) for m in in_maps], axis=0)
        for name in rt["in_names"]]
    prev = rt["prev_outs"]
    if prev is None:
        # replicated output buffers: global shape == per-core shape
        prev = [np.zeros(tuple(av.shape), av.dtype) for av in rt["out_avals"]]
    outs = rt["sharded"](*concat_in, *prev)
    try:
        for o in outs:
            o.copy_to_host_async()
    except Exception:
        pass
    np_outs = [np.asarray(o) for o in outs]
    rt["prev_outs"] = list(outs)
    results = []
    for core in range(8):
        d = {}
        for i, name in enumerate(rt["out_names"]):
            d[name] = np_outs[i][core]  # out[8, P, OUTC]: core's block
        results.append(d)
    return SimpleNamespace(results=results, exec_time_ns=None,
                           profile_json=None, instructions_and_trace=None)


def kernel(**inputs) -> np.ndarray:
    maps = _in_maps(**inputs)
    res = _run(maps).results
    out = np.zeros((B, C, N), np.float32)
    for core in range(8):
        b, h = core // 2, core % 2
        blk = np.asarray(res[core]["out"])                   # [128, OUTC] int8
        o8 = blk[:, :CT * HN].reshape(P, CT, HN)
        osc = blk[:, CT * HN:].copy().view(np.float32)       # [128, CT]
        o = o8.astype(np.float32) * osc[:, :, None]
        out[b][:, h * HN:(h + 1) * HN] = o.transpose(1, 0, 2).reshape(C, HN)
    return out.reshape(B, C, H, W)
